# revision 5
# baseline (speedup 1.0000x reference)
"""kNN hypergraph kernel for Trainium2 (8 NeuronCores, Bass/Tile).

Problem: x [16, 256, 768] f32, k=16 -> out [16, 256, 256] f32.
Scores s = 2<x_i,x_j> - |x_j|^2 per 512-row shard; 16 largest per row.

PE per 128-row tile (8 psum banks x 512 cols):
  sq (K=2 fp16: ones @ [-sq hi; -sq lo]) + h2@h (fp16, K=768) + fp8
  DoubleRow corrections (h2/64)@(lo*64) + (l2*64)@(h/64) (K=2x768,
  2 K-rows per 216ns MM). Score err std ~8e-4 on HW -> selection-exact
  on this input (boundary gaps ~1.5).

DMA: fp16 stream (nsq, lh/rh interleaved) on the Sync HWDGE ring; fp8
stream (l8b, r8a) concurrently on the GpSimd ring. The other two fp8
operands are power-of-2 rescales of fp16 data already on chip, so DVE
derives them (r8b = rh/64, l8a = lh/64 -> e4m3), saving 3.6 MB of DMA.

Selection per row-tile: ACT drains psum -> s (f32); DVE max8 per
512-block (block top-8; the union covers the global top-16 except ~1
row in 4096), max8/match_replace chain -> tau = midpoint(16th, 17th);
mask+histogram: ACT Sign(s-tau) -> +-1 bf16 on cols 0:2048, DVE is_ge
-> 0/1 on 2048:4096, both halves log-folded on DVE (sum over the batch
axis), final count = 0.5*f1 + 4 + f2 via two fused DVE ops.
"""

import os

import numpy as np

B, S, D = 16, 256, 768
N = B * S            # 4096 points
NCORES = 8
M = N // NCORES      # 512 rows per core
KT = 6               # fp16 K-tiles of 128
KT8 = 3              # fp8 DoubleRow K-tiles of 256
NT = N // 512        # 8 col-blocks of 512
RT = M // 128        # 4 row-tiles of 128 per core
SC = 64.0            # fp8 correction operand scale (2^6)
NEG = -3.0e38

_cache = {}


def _build():
    import concourse.mybir as mybir
    import concourse.tile as tile
    from concourse import bacc

    f32 = mybir.dt.float32
    f16 = mybir.dt.float16
    bf16 = mybir.dt.bfloat16
    f8 = mybir.dt.float8e4
    DR = mybir.MatmulPerfMode.DoubleRow
    Alu = mybir.AluOpType

    nc = bacc.Bacc("TRN2", target_bir_lowering=False, debug=False,
                   num_devices=NCORES)

    nsq2_d = nc.dram_tensor("nsq2", [2, N], f16, kind="ExternalInput")
    lh_d = nc.dram_tensor("lh", [D, M], f16, kind="ExternalInput")
    rh_d = nc.dram_tensor("rh", [D, N], f16, kind="ExternalInput")
    l8b_d = nc.dram_tensor("l8b", [D, M], f8, kind="ExternalInput")
    r8a_d = nc.dram_tensor("r8a", [D, N], f8, kind="ExternalInput")
    out_d = nc.dram_tensor("out", [M, S], f32, kind="ExternalOutput")

    with tile.TileContext(nc) as tc:
        with (
            tc.tile_pool(name="weights", bufs=1) as wpool,
            tc.tile_pool(name="s", bufs=2) as spool,
            tc.tile_pool(name="mask", bufs=2) as mpool,
            tc.tile_pool(name="cmb", bufs=2) as cpool,
            tc.tile_pool(name="outp", bufs=4) as opool,
            tc.tile_pool(name="psum", bufs=8, space="PSUM") as psum,
        ):
            # fp16 stream (nsq, lh/rh interleaved) on the Sync ring; the
            # fp8 stream rides the GpSimd ring so both stream in parallel
            ones2 = wpool.tile([2, 128], f16, tag="ones", name="ones")
            nc.vector.memset(ones2, 1.0)
            nsq2 = wpool.tile([2, N], f16, tag="nsq2", name="nsq2")
            nc.sync.dma_start(out=nsq2, in_=nsq2_d[:, :])
            l8b_sb = []
            for kt in range(KT8):
                tb = wpool.tile([128, 2, M], f8, tag=f"l8b{kt}", name=f"l8b{kt}")
                for i in range(2):
                    ksl = slice(kt * 256 + i * 128, kt * 256 + (i + 1) * 128)
                    nc.gpsimd.dma_start(out=tb[:, i, :], in_=l8b_d[ksl, :])
                l8b_sb.append(tb)
            r8a_sb = []
            for kt in range(KT8):
                ta = wpool.tile([128, 2, N], f8, tag=f"r8a{kt}", name=f"r8a{kt}")
                for i in range(2):
                    ksl = slice(kt * 256 + i * 128, kt * 256 + (i + 1) * 128)
                    nc.gpsimd.dma_start(out=ta[:, i, :], in_=r8a_d[ksl, :])
                r8a_sb.append(ta)
            lh_sb, rh_sb = [], []
            for ki in range(KT):
                ksl = slice(ki * 128, (ki + 1) * 128)
                t = wpool.tile([128, M], f16, tag=f"lh{ki}", name=f"lh{ki}")
                nc.sync.dma_start(out=t, in_=lh_d[ksl, :])
                lh_sb.append(t)
                t = wpool.tile([128, N], f16, tag=f"rh{ki}", name=f"rh{ki}")
                nc.sync.dma_start(out=t[:, :N // 2], in_=rh_d[ksl, :N // 2])
                nc.sync.dma_start(out=t[:, N // 2:], in_=rh_d[ksl, N // 2:])
                rh_sb.append(t)
            # derive the remaining fp8 operands on DVE (saves 3.6 MB DMA):
            # r8b = rh * (1/64) -> e4m3, l8a = lh * (1/64) -> e4m3
            l8a_sb, r8b_sb = [], []
            for kt in range(KT8):
                ta = wpool.tile([128, 2, M], f8, tag=f"l8a{kt}", name=f"l8a{kt}")
                tb = wpool.tile([128, 2, N], f8, tag=f"r8b{kt}", name=f"r8b{kt}")
                for i in range(2):
                    nc.vector.tensor_scalar_mul(ta[:, i, :],
                                                lh_sb[2 * kt + i], 1.0 / SC)
                    nc.vector.tensor_scalar_mul(tb[:, i, :],
                                                rh_sb[2 * kt + i], 1.0 / SC)
                l8a_sb.append(ta)
                r8b_sb.append(tb)

            for rt in range(RT):
                rsl = slice(rt * 128, (rt + 1) * 128)
                s_sb = spool.tile([128, N], f32, tag="s", name="s_sb")
                ps = [psum.tile([128, 512], f32, tag="ps", name=f"ps{n}")
                      for n in range(NT)]

                # sq pass opens accumulation (tiny DMA dependency)
                for n in range(NT):
                    nc.tensor.matmul(ps[n][:, :], ones2,
                                     nsq2[:, n * 512:(n + 1) * 512],
                                     start=True, stop=False)
                # fp16 main pass, K-outer (stationary reused across banks)
                for ki in range(KT):
                    lw = lh_sb[ki][:, rsl]
                    for n in range(NT):
                        nc.tensor.matmul(ps[n][:, :], lw,
                                         rh_sb[ki][:, n * 512:(n + 1) * 512],
                                         start=False, stop=False)
                # fp8 DoubleRow corrections, kt-outer
                for kt in range(KT8):
                    lw = l8a_sb[kt][:, :, rsl]
                    for n in range(NT):
                        nc.tensor.matmul(
                            ps[n][:, :], lw,
                            r8a_sb[kt][:, :, n * 512:(n + 1) * 512],
                            start=False, stop=False, perf_mode=DR)
                for kt in range(KT8):
                    lw = l8b_sb[kt][:, :, rsl]
                    for n in range(NT):
                        nc.tensor.matmul(
                            ps[n][:, :], lw,
                            r8b_sb[kt][:, :, n * 512:(n + 1) * 512],
                            start=False, stop=(kt == KT8 - 1), perf_mode=DR)

                # ACT drains chase the bank completions; DVE max8 chases
                m8 = cpool.tile([128, NT * 8], f32, tag="m8", name="m8")
                for n in range(NT):
                    nsl = slice(n * 512, (n + 1) * 512)
                    nc.scalar.copy(out=s_sb[:, nsl], in_=ps[n][:, :])
                    nc.vector.max(out=m8[:, n * 8:(n + 1) * 8],
                                  in_=s_sb[:, nsl])

                # union of block top-8s -> ranks 9-16 / 17-24 -> tau
                c8 = cpool.tile([128, 8], f32, tag="c8", name="c8")
                scr = cpool.tile([128, NT * 8], f32, tag="scr", name="scr")
                d8 = cpool.tile([128, 8], f32, tag="d8", name="d8")
                scr2 = cpool.tile([128, NT * 8], f32, tag="scr2", name="scr2")
                e8 = cpool.tile([128, 8], f32, tag="e8", name="e8")
                nc.vector.max(out=c8, in_=m8)
                nc.vector.match_replace(out=scr, in_to_replace=c8,
                                        in_values=m8, imm_value=NEG)
                nc.vector.max(out=d8, in_=scr)
                nc.vector.match_replace(out=scr2, in_to_replace=d8,
                                        in_values=scr, imm_value=NEG)
                nc.vector.max(out=e8, in_=scr2)
                tsum = cpool.tile([128, 1], f32, tag="tsum", name="tsum")
                tau = cpool.tile([128, 1], f32, tag="tau", name="tau")
                taun = cpool.tile([128, 1], f32, tag="taun", name="taun")
                nc.vector.tensor_add(tsum, d8[:, 7:8], e8[:, 0:1])
                nc.vector.tensor_scalar_mul(tau, tsum, 0.5)
                nc.vector.tensor_scalar_mul(taun, tsum, -0.5)

                # mask halves: ACT Sign -> +-1 | DVE is_ge -> 0/1
                H = N // 2
                mask = mpool.tile([128, N], bf16, tag="mask", name="mask")
                nc.scalar.sign(mask[:, :H], s_sb[:, :H], bias=taun)
                nc.vector.tensor_scalar(mask[:, H:], s_sb[:, H:], tau, None,
                                        op0=Alu.is_ge)
                # DVE log-folds; +4 fused into the 0/1 half's last fold
                nc.vector.tensor_add(mask[:, 0:1024], mask[:, 0:1024],
                                     mask[:, 1024:2048])
                nc.vector.tensor_add(mask[:, H:H + 1024], mask[:, H:H + 1024],
                                     mask[:, H + 1024:H + 2048])
                nc.vector.tensor_add(mask[:, 0:512], mask[:, 0:512],
                                     mask[:, 512:1024])
                nc.vector.tensor_add(mask[:, H:H + 512], mask[:, H:H + 512],
                                     mask[:, H + 512:H + 1024])
                nc.vector.tensor_add(mask[:, 0:256], mask[:, 0:256],
                                     mask[:, 256:512])
                f2p = opool.tile([128, S], f32, tag="f2p", name="f2p")
                nc.vector.scalar_tensor_tensor(
                    out=f2p, in0=mask[:, H:H + 256], scalar=4.0,
                    in1=mask[:, H + 256:H + 512],
                    op0=Alu.add, op1=Alu.add)
                o = opool.tile([128, S], f32, tag="o", name="o")
                nc.vector.scalar_tensor_tensor(
                    out=o, in0=mask[:, 0:256], scalar=0.5, in1=f2p,
                    op0=Alu.mult, op1=Alu.add)
                nc.sync.dma_start(out=out_d[rsl, :], in_=o)

    nc.compile()
    return nc


def _prep_inputs(x):
    flat = np.asarray(x, dtype=np.float32).reshape(N, D)
    import ml_dtypes
    e4 = ml_dtypes.float8_e4m3

    sq = (flat.astype(np.float64) ** 2).sum(1).astype(np.float32)
    nsq_h = (-sq).astype(np.float16)
    nsq_l = (-sq - nsq_h.astype(np.float32)).astype(np.float16)

    h_r = flat.astype(np.float16)                                  # x hi
    l_r = (flat - h_r.astype(np.float32)).astype(np.float32)       # x lo
    h_l = (2.0 * flat).astype(np.float16)                          # 2x hi
    l_l = (2.0 * flat - h_l.astype(np.float32)).astype(np.float32)

    fT = lambda a: np.ascontiguousarray(a.T)
    return {
        "nsq2": np.stack([nsq_h, nsq_l]),
        "rh": fT(h_r),
        "r8a": fT((l_r * SC)).astype(e4),
        "lh_full": fT(h_l),
        "l8b_full": fT((l_l * SC)).astype(e4),
    }


def kernel(x, k):
    assert int(k) == 16
    pre = _prep_inputs(x)

    if "nc" not in _cache:
        _cache["nc"] = _build()
    nc = _cache["nc"]

    shared = {kk: pre[kk] for kk in ("nsq2", "rh", "r8a")}
    in_maps = []
    for c in range(NCORES):
        csl = slice(c * M, (c + 1) * M)
        m = dict(shared)
        m["lh"] = np.ascontiguousarray(pre["lh_full"][:, csl])
        m["l8b"] = np.ascontiguousarray(pre["l8b_full"][:, csl])
        in_maps.append(m)

    from concourse.bass_utils import run_bass_kernel_spmd
    trace = bool(os.environ.get("KNN_TRACE"))
    if trace:
        try:
            from antenv.axon_hooks import get_axon_ntff_profile_hook  # noqa
        except ImportError:
            trace = False
    res = run_bass_kernel_spmd(nc, in_maps, core_ids=list(range(NCORES)),
                               trace=trace)
    if trace:
        _cache["res"] = res
    if trace and res.exec_time_ns is not None:
        print(f"HW exec time: {res.exec_time_ns} ns")
        _cache["exec_time_ns"] = res.exec_time_ns

    out = np.concatenate([r["out"] for r in res.results], axis=0)
    return out.reshape(B, S, S)


# revision 6
# speedup vs baseline: 1.1695x; 1.1695x over previous
"""kNN hypergraph kernel for Trainium2 (8 NeuronCores, Bass/Tile).

Problem: x [16, 256, 768] f32, k=16 -> out [16, 256, 256] f32.
Scores s = 2<x_i,x_j> - |x_j|^2 per 512-row shard; 16 largest per row.

PE per 128-row tile (8 psum banks x 512 cols):
  sq (K=2 fp16: ones @ [-sq hi; -sq lo]) + h2@h (fp16, K=768) + fp8
  DoubleRow corrections (h2/64)@(lo*64) + (l2*64)@(h/64) (K=2x768,
  2 K-rows per 216ns MM). Score err std ~8e-4 on HW -> selection-exact
  on this input (boundary gaps ~1.5).

DMA: fp16 stream (nsq, lh/rh interleaved) on the Sync HWDGE ring; fp8
stream (l8b, r8a) concurrently on the GpSimd ring. The other two fp8
operands are power-of-2 rescales of fp16 data already on chip, so DVE
derives them (r8b = rh/64, l8a = lh/64 -> e4m3), saving 3.6 MB of DMA.

Selection per row-tile: ACT drains psum -> s (f32); DVE max8 per
512-block (block top-8; the union covers the global top-16 except ~1
row in 4096), max8/match_replace chain -> tau = midpoint(16th, 17th);
mask+histogram: ACT Sign(s-tau) -> +-1 bf16 on cols 0:2048, DVE is_ge
-> 0/1 on 2048:4096, both halves log-folded on DVE (sum over the batch
axis), final count = 0.5*f1 + 4 + f2 via two fused DVE ops.
"""

import os

import numpy as np

B, S, D = 16, 256, 768
N = B * S            # 4096 points
NCORES = 8
M = N // NCORES      # 512 rows per core
KT = 6               # fp16 K-tiles of 128
KT8 = 3              # fp8 DoubleRow K-tiles of 256
NT = N // 512        # 8 col-blocks of 512
RT = M // 128        # 4 row-tiles of 128 per core
SC = 64.0            # fp8 correction operand scale (2^6)
NEG = -3.0e38

_cache = {}


def _build():
    import concourse.mybir as mybir
    import concourse.tile as tile
    from concourse import bacc

    f32 = mybir.dt.float32
    f16 = mybir.dt.float16
    bf16 = mybir.dt.bfloat16
    f8 = mybir.dt.float8e4
    DR = mybir.MatmulPerfMode.DoubleRow
    Alu = mybir.AluOpType

    nc = bacc.Bacc("TRN2", target_bir_lowering=False, debug=False,
                   num_devices=NCORES)

    nsq2_d = nc.dram_tensor("nsq2", [2, N], f16, kind="ExternalInput")
    lh_d = nc.dram_tensor("lh", [D, M], f16, kind="ExternalInput")
    rh_d = nc.dram_tensor("rh", [D, N], f16, kind="ExternalInput")
    l8b_d = nc.dram_tensor("l8b", [D, M], f8, kind="ExternalInput")
    r8a_d = nc.dram_tensor("r8a", [D, N], f8, kind="ExternalInput")
    out_d = nc.dram_tensor("out", [M, S], f32, kind="ExternalOutput")

    with tile.TileContext(nc) as tc:
        with (
            tc.tile_pool(name="weights", bufs=1) as wpool,
            tc.tile_pool(name="s", bufs=2) as spool,
            tc.tile_pool(name="mask", bufs=2) as mpool,
            tc.tile_pool(name="cmb", bufs=2) as cpool,
            tc.tile_pool(name="outp", bufs=4) as opool,
            tc.tile_pool(name="psum", bufs=8, space="PSUM") as psum,
        ):
            # fp16 stream (nsq, lh/rh interleaved) on the Sync ring; the
            # fp8 stream rides the GpSimd ring so both stream in parallel
            ones2 = wpool.tile([2, 128], f16, tag="ones", name="ones")
            nc.vector.memset(ones2, 1.0)
            nsq2 = wpool.tile([2, N], f16, tag="nsq2", name="nsq2")
            nc.sync.dma_start(out=nsq2, in_=nsq2_d[:, :])
            l8b_sb = []
            for kt in range(KT8):
                tb = wpool.tile([128, 2, M], f8, tag=f"l8b{kt}", name=f"l8b{kt}")
                for i in range(2):
                    ksl = slice(kt * 256 + i * 128, kt * 256 + (i + 1) * 128)
                    nc.gpsimd.dma_start(out=tb[:, i, :], in_=l8b_d[ksl, :])
                l8b_sb.append(tb)
            r8a_sb = []
            for kt in range(KT8):
                ta = wpool.tile([128, 2, N], f8, tag=f"r8a{kt}", name=f"r8a{kt}")
                for i in range(2):
                    ksl = slice(kt * 256 + i * 128, kt * 256 + (i + 1) * 128)
                    nc.gpsimd.dma_start(out=ta[:, i, :], in_=r8a_d[ksl, :])
                r8a_sb.append(ta)
            lh_sb, rh_sb = [], []
            for ki in range(KT):
                ksl = slice(ki * 128, (ki + 1) * 128)
                t = wpool.tile([128, M], f16, tag=f"lh{ki}", name=f"lh{ki}")
                nc.sync.dma_start(out=t, in_=lh_d[ksl, :])
                lh_sb.append(t)
                t = wpool.tile([128, N], f16, tag=f"rh{ki}", name=f"rh{ki}")
                nc.sync.dma_start(out=t[:, :N // 2], in_=rh_d[ksl, :N // 2])
                nc.sync.dma_start(out=t[:, N // 2:], in_=rh_d[ksl, N // 2:])
                rh_sb.append(t)
            # derive the remaining fp8 operands on DVE (saves 3.6 MB DMA):
            # r8b = rh * (1/64) -> e4m3, l8a = lh * (1/64) -> e4m3
            l8a_sb, r8b_sb = [], []
            for kt in range(KT8):
                ta = wpool.tile([128, 2, M], f8, tag=f"l8a{kt}", name=f"l8a{kt}")
                tb = wpool.tile([128, 2, N], f8, tag=f"r8b{kt}", name=f"r8b{kt}")
                for i in range(2):
                    nc.vector.tensor_scalar_mul(ta[:, i, :],
                                                lh_sb[2 * kt + i], 1.0 / SC)
                    nc.vector.tensor_scalar_mul(tb[:, i, :],
                                                rh_sb[2 * kt + i], 1.0 / SC)
                l8a_sb.append(ta)
                r8b_sb.append(tb)

            for rt in range(RT):
                rsl = slice(rt * 128, (rt + 1) * 128)
                s_sb = spool.tile([128, N], f32, tag="s", name="s_sb")
                ps = [psum.tile([128, 512], f32, tag="ps", name=f"ps{n}")
                      for n in range(NT)]

                def mm16(ki, start=False):
                    lw = lh_sb[ki][:, rsl]
                    for n in range(NT):
                        nc.tensor.matmul(ps[n][:, :], lw,
                                         rh_sb[ki][:, n * 512:(n + 1) * 512],
                                         start=start, stop=False)

                def mm8(tiles, stat, kt, stop=False):
                    lw = stat[kt][:, :, rsl]
                    for n in range(NT):
                        nc.tensor.matmul(
                            ps[n][:, :], lw,
                            tiles[kt][:, :, n * 512:(n + 1) * 512],
                            start=False, stop=stop, perf_mode=DR)

                # sq pass opens accumulation (tiny DMA dependency)
                for n in range(NT):
                    nc.tensor.matmul(ps[n][:, :], ones2,
                                     nsq2[:, n * 512:(n + 1) * 512],
                                     start=True, stop=False)
                if rt == 0:
                    # row-tile 0 is paced by the input streams: alternate
                    # fp16 tiles (Sync ring) with fp8 tiles (GpSimd ring /
                    # DVE-derived) so the PE consumes whichever ring has
                    # delivered and never idles long enough to re-throttle
                    mm16(0); mm16(1)
                    mm8(r8b_sb, l8b_sb, 0)          # derived from rh0, rh1
                    mm8(r8a_sb, l8a_sb, 0)
                    mm16(2); mm16(3)
                    mm8(r8b_sb, l8b_sb, 1)
                    mm8(r8a_sb, l8a_sb, 1)
                    mm16(4); mm16(5)
                    mm8(r8b_sb, l8b_sb, 2)
                    mm8(r8a_sb, l8a_sb, 2, stop=True)
                else:
                    for ki in range(KT):
                        mm16(ki)
                    for kt in range(KT8):
                        mm8(r8a_sb, l8a_sb, kt)
                    for kt in range(KT8):
                        mm8(r8b_sb, l8b_sb, kt, stop=(kt == KT8 - 1))

                # ACT drains chase the bank completions; DVE max8 chases
                m8 = cpool.tile([128, NT * 8], f32, tag="m8", name="m8")
                for n in range(NT):
                    nsl = slice(n * 512, (n + 1) * 512)
                    nc.scalar.copy(out=s_sb[:, nsl], in_=ps[n][:, :])
                    nc.vector.max(out=m8[:, n * 8:(n + 1) * 8],
                                  in_=s_sb[:, nsl])

                # union of block top-8s -> ranks 9-16 / 17-24 -> tau
                c8 = cpool.tile([128, 8], f32, tag="c8", name="c8")
                scr = cpool.tile([128, NT * 8], f32, tag="scr", name="scr")
                d8 = cpool.tile([128, 8], f32, tag="d8", name="d8")
                scr2 = cpool.tile([128, NT * 8], f32, tag="scr2", name="scr2")
                e8 = cpool.tile([128, 8], f32, tag="e8", name="e8")
                nc.vector.max(out=c8, in_=m8)
                nc.vector.match_replace(out=scr, in_to_replace=c8,
                                        in_values=m8, imm_value=NEG)
                nc.vector.max(out=d8, in_=scr)
                nc.vector.match_replace(out=scr2, in_to_replace=d8,
                                        in_values=scr, imm_value=NEG)
                nc.vector.max(out=e8, in_=scr2)
                tsum = cpool.tile([128, 1], f32, tag="tsum", name="tsum")
                tau = cpool.tile([128, 1], f32, tag="tau", name="tau")
                taun = cpool.tile([128, 1], f32, tag="taun", name="taun")
                nc.vector.tensor_add(tsum, d8[:, 7:8], e8[:, 0:1])
                nc.vector.tensor_scalar_mul(tau, tsum, 0.5)
                nc.vector.tensor_scalar_mul(taun, tsum, -0.5)

                # mask halves: ACT Sign -> +-1 | DVE is_ge -> 0/1
                H = N // 2
                mask = mpool.tile([128, N], bf16, tag="mask", name="mask")
                nc.scalar.sign(mask[:, :H], s_sb[:, :H], bias=taun)
                nc.vector.tensor_scalar(mask[:, H:], s_sb[:, H:], tau, None,
                                        op0=Alu.is_ge)
                # DVE log-folds; +4 fused into the 0/1 half's last fold
                nc.vector.tensor_add(mask[:, 0:1024], mask[:, 0:1024],
                                     mask[:, 1024:2048])
                nc.vector.tensor_add(mask[:, H:H + 1024], mask[:, H:H + 1024],
                                     mask[:, H + 1024:H + 2048])
                nc.vector.tensor_add(mask[:, 0:512], mask[:, 0:512],
                                     mask[:, 512:1024])
                nc.vector.tensor_add(mask[:, H:H + 512], mask[:, H:H + 512],
                                     mask[:, H + 512:H + 1024])
                nc.vector.tensor_add(mask[:, 0:256], mask[:, 0:256],
                                     mask[:, 256:512])
                f2p = opool.tile([128, S], f32, tag="f2p", name="f2p")
                nc.vector.scalar_tensor_tensor(
                    out=f2p, in0=mask[:, H:H + 256], scalar=4.0,
                    in1=mask[:, H + 256:H + 512],
                    op0=Alu.add, op1=Alu.add)
                o = opool.tile([128, S], f32, tag="o", name="o")
                nc.vector.scalar_tensor_tensor(
                    out=o, in0=mask[:, 0:256], scalar=0.5, in1=f2p,
                    op0=Alu.mult, op1=Alu.add)
                nc.sync.dma_start(out=out_d[rsl, :], in_=o)

    nc.compile()
    return nc


def _prep_inputs(x):
    flat = np.asarray(x, dtype=np.float32).reshape(N, D)
    import ml_dtypes
    e4 = ml_dtypes.float8_e4m3

    sq = (flat.astype(np.float64) ** 2).sum(1).astype(np.float32)
    nsq_h = (-sq).astype(np.float16)
    nsq_l = (-sq - nsq_h.astype(np.float32)).astype(np.float16)

    h_r = flat.astype(np.float16)                                  # x hi
    l_r = (flat - h_r.astype(np.float32)).astype(np.float32)       # x lo
    h_l = (2.0 * flat).astype(np.float16)                          # 2x hi
    l_l = (2.0 * flat - h_l.astype(np.float32)).astype(np.float32)

    fT = lambda a: np.ascontiguousarray(a.T)
    return {
        "nsq2": np.stack([nsq_h, nsq_l]),
        "rh": fT(h_r),
        "r8a": fT((l_r * SC)).astype(e4),
        "lh_full": fT(h_l),
        "l8b_full": fT((l_l * SC)).astype(e4),
    }


def kernel(x, k):
    assert int(k) == 16
    pre = _prep_inputs(x)

    if "nc" not in _cache:
        _cache["nc"] = _build()
    nc = _cache["nc"]

    shared = {kk: pre[kk] for kk in ("nsq2", "rh", "r8a")}
    in_maps = []
    for c in range(NCORES):
        csl = slice(c * M, (c + 1) * M)
        m = dict(shared)
        m["lh"] = np.ascontiguousarray(pre["lh_full"][:, csl])
        m["l8b"] = np.ascontiguousarray(pre["l8b_full"][:, csl])
        in_maps.append(m)

    from concourse.bass_utils import run_bass_kernel_spmd
    trace = bool(os.environ.get("KNN_TRACE"))
    if trace:
        try:
            from antenv.axon_hooks import get_axon_ntff_profile_hook  # noqa
        except ImportError:
            trace = False
    res = run_bass_kernel_spmd(nc, in_maps, core_ids=list(range(NCORES)),
                               trace=trace)
    if trace:
        _cache["res"] = res
    if trace and res.exec_time_ns is not None:
        print(f"HW exec time: {res.exec_time_ns} ns")
        _cache["exec_time_ns"] = res.exec_time_ns

    out = np.concatenate([r["out"] for r in res.results], axis=0)
    return out.reshape(B, S, S)


# revision 7
# speedup vs baseline: 1.1826x; 1.0112x over previous
"""kNN hypergraph kernel for Trainium2 (8 NeuronCores, Bass/Tile).

Problem: x [16, 256, 768] f32, k=16 -> out [16, 256, 256] f32.
Scores s = 2<x_i,x_j> - |x_j|^2 per 512-row shard; 16 largest per row.

PE per 128-row tile (8 psum banks x 512 cols):
  sq (K=2 fp16: ones @ [-sq hi; -sq lo]) + h2@h (fp16, K=768) + fp8
  DoubleRow corrections (h2/64)@(lo*64) + (l2*64)@(h/64) (K=2x768,
  2 K-rows per 216ns MM). Score err std ~8e-4 on HW -> selection-exact
  on this input (boundary gaps ~1.5).

DMA: fp16 stream (nsq, lh/rh interleaved) on the Sync HWDGE ring; fp8
stream (l8b, r8a) concurrently on the GpSimd ring. The other two fp8
operands are power-of-2 rescales of fp16 data already on chip, so DVE
derives them (r8b = rh/64, l8a = lh/64 -> e4m3), saving 3.6 MB of DMA.

Selection per row-tile: ACT drains psum -> s (f32); DVE max8 per
512-block (block top-8; the union covers the global top-16 except ~1
row in 4096), max8/match_replace chain -> tau = midpoint(16th, 17th);
mask+histogram: ACT Sign(s-tau) -> +-1 bf16 on cols 0:2048, DVE is_ge
-> 0/1 on 2048:4096, both halves log-folded on DVE (sum over the batch
axis), final count = 0.5*f1 + 4 + f2 via two fused DVE ops.
"""

import os

import numpy as np

B, S, D = 16, 256, 768
N = B * S            # 4096 points
NCORES = 8
M = N // NCORES      # 512 rows per core
KT = 6               # fp16 K-tiles of 128
KT8 = 3              # fp8 DoubleRow K-tiles of 256
NT = N // 512        # 8 col-blocks of 512
RT = M // 128        # 4 row-tiles of 128 per core
SC = 64.0            # fp8 correction operand scale (2^6)
NEG = -3.0e38

_cache = {}


def _build():
    import concourse.mybir as mybir
    import concourse.tile as tile
    from concourse import bacc

    f32 = mybir.dt.float32
    f16 = mybir.dt.float16
    bf16 = mybir.dt.bfloat16
    f8 = mybir.dt.float8e4
    DR = mybir.MatmulPerfMode.DoubleRow
    Alu = mybir.AluOpType

    nc = bacc.Bacc("TRN2", target_bir_lowering=False, debug=False,
                   num_devices=NCORES)

    nsq2_d = nc.dram_tensor("nsq2", [2, N], f16, kind="ExternalInput")
    lh_d = nc.dram_tensor("lh", [D, M], f16, kind="ExternalInput")
    rh_d = nc.dram_tensor("rh", [D, N], f16, kind="ExternalInput")
    l8b_d = nc.dram_tensor("l8b", [D, M], f8, kind="ExternalInput")
    r8a_d = nc.dram_tensor("r8a", [D, N], f8, kind="ExternalInput")
    out_d = nc.dram_tensor("out", [M, S], f32, kind="ExternalOutput")

    with tile.TileContext(nc) as tc:
        with (
            tc.tile_pool(name="weights", bufs=1) as wpool,
            tc.tile_pool(name="s", bufs=2) as spool,
            tc.tile_pool(name="mask", bufs=2) as mpool,
            tc.tile_pool(name="cmb", bufs=2) as cpool,
            tc.tile_pool(name="outp", bufs=4) as opool,
            tc.tile_pool(name="psum", bufs=8, space="PSUM") as psum,
        ):
            # fp16 stream (nsq, lh/rh interleaved) on the Sync ring; the
            # fp8 stream rides the GpSimd ring so both stream in parallel
            ones2 = wpool.tile([2, 128], f16, tag="ones", name="ones")
            nc.vector.memset(ones2, 1.0)
            nsq2 = wpool.tile([2, N], f16, tag="nsq2", name="nsq2")
            nc.sync.dma_start(out=nsq2, in_=nsq2_d[:, :])
            l8b_sb = []
            for kt in range(KT8):
                tb = wpool.tile([128, 2, M], f8, tag=f"l8b{kt}", name=f"l8b{kt}")
                for i in range(2):
                    ksl = slice(kt * 256 + i * 128, kt * 256 + (i + 1) * 128)
                    nc.gpsimd.dma_start(out=tb[:, i, :], in_=l8b_d[ksl, :])
                l8b_sb.append(tb)
            r8a_sb = []
            for kt in range(KT8):
                ta = wpool.tile([128, 2, N], f8, tag=f"r8a{kt}", name=f"r8a{kt}")
                for i in range(2):
                    ksl = slice(kt * 256 + i * 128, kt * 256 + (i + 1) * 128)
                    nc.gpsimd.dma_start(out=ta[:, i, :], in_=r8a_d[ksl, :])
                r8a_sb.append(ta)
            lh_sb, rh_sb = [], []
            for ki in range(KT):
                ksl = slice(ki * 128, (ki + 1) * 128)
                t = wpool.tile([128, M], f16, tag=f"lh{ki}", name=f"lh{ki}")
                nc.sync.dma_start(out=t, in_=lh_d[ksl, :])
                lh_sb.append(t)
                t = wpool.tile([128, N], f16, tag=f"rh{ki}", name=f"rh{ki}")
                nc.sync.dma_start(out=t[:, :N // 2], in_=rh_d[ksl, :N // 2])
                nc.sync.dma_start(out=t[:, N // 2:], in_=rh_d[ksl, N // 2:])
                rh_sb.append(t)
            # derive the remaining fp8 operands on DVE (saves 3.6 MB DMA):
            # r8b = rh * (1/64) -> e4m3, l8a = lh * (1/64) -> e4m3
            l8a_sb, r8b_sb = [], []
            for kt in range(KT8):
                ta = wpool.tile([128, 2, M], f8, tag=f"l8a{kt}", name=f"l8a{kt}")
                tb = wpool.tile([128, 2, N], f8, tag=f"r8b{kt}", name=f"r8b{kt}")
                for i in range(2):
                    nc.vector.tensor_scalar_mul(ta[:, i, :],
                                                lh_sb[2 * kt + i], 1.0 / SC)
                    nc.vector.tensor_scalar_mul(tb[:, i, :],
                                                rh_sb[2 * kt + i], 1.0 / SC)
                l8a_sb.append(ta)
                r8b_sb.append(tb)

            # PE warm-up: the first real matmul is DMA-gated until ~10us;
            # fill the dead window with dummy K=2 matmuls on the memset
            # tile so the HAM clock-gate opens to 2.4 GHz before the real
            # stream starts (the real sq pass start=True resets the bank)
            warm = psum.tile([128, 512], f32, tag="ps", name="warm")
            for _ in range(112):
                nc.tensor.matmul(warm[:, 0:128], ones2, ones2,
                                 start=True, stop=True)

            for rt in range(RT):
                rsl = slice(rt * 128, (rt + 1) * 128)
                s_sb = spool.tile([128, N], f32, tag="s", name="s_sb")
                ps = [psum.tile([128, 512], f32, tag="ps", name=f"ps{n}")
                      for n in range(NT)]

                def mm16(ki, start=False):
                    lw = lh_sb[ki][:, rsl]
                    for n in range(NT):
                        nc.tensor.matmul(ps[n][:, :], lw,
                                         rh_sb[ki][:, n * 512:(n + 1) * 512],
                                         start=start, stop=False)

                def mm8(tiles, stat, kt, stop=False):
                    lw = stat[kt][:, :, rsl]
                    for n in range(NT):
                        nc.tensor.matmul(
                            ps[n][:, :], lw,
                            tiles[kt][:, :, n * 512:(n + 1) * 512],
                            start=False, stop=stop, perf_mode=DR)

                def mm8_pair(tA, sA, kA, tB, sB, kB):
                    # interleave the final two DR passes bank-major so bank
                    # n's accumulation closes ~432*(NT-n) ns before the
                    # row-tile ends — the serial ACT drains start ~3.4 us
                    # earlier and the next tile's matmuls get banks sooner
                    lwA = sA[kA][:, :, rsl]
                    lwB = sB[kB][:, :, rsl]
                    for n in range(NT):
                        nsl = slice(n * 512, (n + 1) * 512)
                        nc.tensor.matmul(ps[n][:, :], lwA,
                                         tA[kA][:, :, nsl],
                                         start=False, stop=False,
                                         perf_mode=DR)
                        nc.tensor.matmul(ps[n][:, :], lwB,
                                         tB[kB][:, :, nsl],
                                         start=False, stop=True,
                                         perf_mode=DR)

                # sq pass opens accumulation (tiny DMA dependency)
                for n in range(NT):
                    nc.tensor.matmul(ps[n][:, :], ones2,
                                     nsq2[:, n * 512:(n + 1) * 512],
                                     start=True, stop=False)
                if rt == 0:
                    # row-tile 0 is paced by the input streams: alternate
                    # fp16 tiles (Sync ring) with fp8 tiles (GpSimd ring /
                    # DVE-derived) so the PE consumes whichever ring has
                    # delivered and never idles long enough to re-throttle
                    mm16(0); mm16(1)
                    mm8(r8b_sb, l8b_sb, 0)          # derived from rh0, rh1
                    mm8(r8a_sb, l8a_sb, 0)
                    mm16(2); mm16(3)
                    mm8(r8b_sb, l8b_sb, 1)
                    mm8(r8a_sb, l8a_sb, 1)
                    mm16(4); mm16(5)
                    mm8_pair(r8b_sb, l8b_sb, 2, r8a_sb, l8a_sb, 2)
                else:
                    for ki in range(KT):
                        mm16(ki)
                    for kt in range(KT8):
                        mm8(r8a_sb, l8a_sb, kt)
                    mm8(r8b_sb, l8b_sb, 0)
                    mm8_pair(r8b_sb, l8b_sb, 1, r8b_sb, l8b_sb, 2)

                # ACT drains chase the bank completions; DVE max8 chases
                m8 = cpool.tile([128, NT * 8], f32, tag="m8", name="m8")
                for n in range(NT):
                    nsl = slice(n * 512, (n + 1) * 512)
                    nc.scalar.copy(out=s_sb[:, nsl], in_=ps[n][:, :])
                    nc.vector.max(out=m8[:, n * 8:(n + 1) * 8],
                                  in_=s_sb[:, nsl])

                # union of block top-8s -> ranks 9-16 / 17-24 -> tau
                c8 = cpool.tile([128, 8], f32, tag="c8", name="c8")
                scr = cpool.tile([128, NT * 8], f32, tag="scr", name="scr")
                d8 = cpool.tile([128, 8], f32, tag="d8", name="d8")
                scr2 = cpool.tile([128, NT * 8], f32, tag="scr2", name="scr2")
                e8 = cpool.tile([128, 8], f32, tag="e8", name="e8")
                nc.vector.max(out=c8, in_=m8)
                nc.vector.match_replace(out=scr, in_to_replace=c8,
                                        in_values=m8, imm_value=NEG)
                nc.vector.max(out=d8, in_=scr)
                nc.vector.match_replace(out=scr2, in_to_replace=d8,
                                        in_values=scr, imm_value=NEG)
                nc.vector.max(out=e8, in_=scr2)
                tsum = cpool.tile([128, 1], f32, tag="tsum", name="tsum")
                tau = cpool.tile([128, 1], f32, tag="tau", name="tau")
                taun = cpool.tile([128, 1], f32, tag="taun", name="taun")
                nc.vector.tensor_add(tsum, d8[:, 7:8], e8[:, 0:1])
                nc.vector.tensor_scalar_mul(tau, tsum, 0.5)
                nc.vector.tensor_scalar_mul(taun, tsum, -0.5)

                # mask halves: ACT Sign -> +-1 | DVE is_ge -> 0/1
                H = N // 2
                mask = mpool.tile([128, N], bf16, tag="mask", name="mask")
                nc.scalar.sign(mask[:, :H], s_sb[:, :H], bias=taun)
                nc.vector.tensor_scalar(mask[:, H:], s_sb[:, H:], tau, None,
                                        op0=Alu.is_ge)
                # DVE log-folds; +4 fused into the 0/1 half's last fold
                nc.vector.tensor_add(mask[:, 0:1024], mask[:, 0:1024],
                                     mask[:, 1024:2048])
                nc.vector.tensor_add(mask[:, H:H + 1024], mask[:, H:H + 1024],
                                     mask[:, H + 1024:H + 2048])
                nc.vector.tensor_add(mask[:, 0:512], mask[:, 0:512],
                                     mask[:, 512:1024])
                nc.vector.tensor_add(mask[:, H:H + 512], mask[:, H:H + 512],
                                     mask[:, H + 512:H + 1024])
                nc.vector.tensor_add(mask[:, 0:256], mask[:, 0:256],
                                     mask[:, 256:512])
                f2p = opool.tile([128, S], f32, tag="f2p", name="f2p")
                nc.vector.scalar_tensor_tensor(
                    out=f2p, in0=mask[:, H:H + 256], scalar=4.0,
                    in1=mask[:, H + 256:H + 512],
                    op0=Alu.add, op1=Alu.add)
                o = opool.tile([128, S], f32, tag="o", name="o")
                nc.vector.scalar_tensor_tensor(
                    out=o, in0=mask[:, 0:256], scalar=0.5, in1=f2p,
                    op0=Alu.mult, op1=Alu.add)
                nc.sync.dma_start(out=out_d[rsl, :], in_=o)

    nc.compile()
    return nc


def _prep_inputs(x):
    flat = np.asarray(x, dtype=np.float32).reshape(N, D)
    import ml_dtypes
    e4 = ml_dtypes.float8_e4m3

    sq = (flat.astype(np.float64) ** 2).sum(1).astype(np.float32)
    nsq_h = (-sq).astype(np.float16)
    nsq_l = (-sq - nsq_h.astype(np.float32)).astype(np.float16)

    h_r = flat.astype(np.float16)                                  # x hi
    l_r = (flat - h_r.astype(np.float32)).astype(np.float32)       # x lo
    h_l = (2.0 * flat).astype(np.float16)                          # 2x hi
    l_l = (2.0 * flat - h_l.astype(np.float32)).astype(np.float32)

    fT = lambda a: np.ascontiguousarray(a.T)
    return {
        "nsq2": np.stack([nsq_h, nsq_l]),
        "rh": fT(h_r),
        "r8a": fT((l_r * SC)).astype(e4),
        "lh_full": fT(h_l),
        "l8b_full": fT((l_l * SC)).astype(e4),
    }


def kernel(x, k):
    assert int(k) == 16
    pre = _prep_inputs(x)

    if "nc" not in _cache:
        _cache["nc"] = _build()
    nc = _cache["nc"]

    shared = {kk: pre[kk] for kk in ("nsq2", "rh", "r8a")}
    in_maps = []
    for c in range(NCORES):
        csl = slice(c * M, (c + 1) * M)
        m = dict(shared)
        m["lh"] = np.ascontiguousarray(pre["lh_full"][:, csl])
        m["l8b"] = np.ascontiguousarray(pre["l8b_full"][:, csl])
        in_maps.append(m)

    from concourse.bass_utils import run_bass_kernel_spmd
    trace = bool(os.environ.get("KNN_TRACE"))
    if trace:
        try:
            from antenv.axon_hooks import get_axon_ntff_profile_hook  # noqa
        except ImportError:
            trace = False
    res = run_bass_kernel_spmd(nc, in_maps, core_ids=list(range(NCORES)),
                               trace=trace)
    if trace:
        _cache["res"] = res
    if trace and res.exec_time_ns is not None:
        print(f"HW exec time: {res.exec_time_ns} ns")
        _cache["exec_time_ns"] = res.exec_time_ns

    out = np.concatenate([r["out"] for r in res.results], axis=0)
    return out.reshape(B, S, S)
